# revision 1
# baseline (speedup 1.0000x reference)
"""CRF negative-log-likelihood loss on 8 Trainium2 NeuronCores.

Strategy: data-parallel over batch (128 sequences per core). The forward
(log-partition) recurrence is run on device in the exp domain so each time
step is one PE matmul + one DVE multiply:

    W_{t+1} = (E' @ W_t) * exp(logits_t - C0),   E' = exp(transitions)

with state W laid out [K=52 partitions, batch free]. The partition value
q_t = r^T W_t (r = exp(transitions[STOP])) is produced by a second tiny
matmul that writes directly into PSUM row (t mod 128); each 128-row block
is DMA'd to HBM. Every RS steps W is rescaled by 1/q_t (DVE reciprocal +
PE broadcast) to stay in fp32 range; the host reconstructs the log-scale
offsets exactly from the stored q stream. Gold-path (emission/transition)
scores are cheap gathers done host-side during the gather/unshard step.
"""

import numpy as np

# problem constants (hardcoded per contract)
B, T, K = 1024, 512, 52
START, STOP = 50, 51
NCORES = 8
BPC = B // NCORES          # 128 batch per core
C0 = 5.0                   # per-step constant log-shift folded into exp(logits)
RS = 32                    # rescale period
CH = 32                    # time steps per DMA chunk
NBLK = T // 128 + 1        # 5 q-blocks (t = 0..512)
RROW = 64                  # PSUM partition holding q (legal AP start)

_PROG_CACHE = {}


def _build_program(G):
    """Build + compile the SPMD bass program. G = independent batch groups
    per core (interleaved chains to hide serial-dependence latency)."""
    import concourse.mybir as mybir
    import concourse.tile as tile
    from concourse import bacc

    f32 = mybir.dt.float32
    BG = BPC // G

    nc = bacc.Bacc("TRN2", target_bir_lowering=False, debug=False,
                   num_devices=NCORES)
    expLT_d = nc.dram_tensor("expLT", [K, T, BPC], f32, kind="ExternalInput")
    # ehatT columns: 0..51 = E' rows, 52..63 = zero pad, 64 = r row.
    # (Engine APs must start at partition 0/32/64/96, so q lives at
    # PSUM partition 64 where it is legally addressable.)
    ehatT_d = nc.dram_tensor("ehatT", [K, RROW + 1], f32, kind="ExternalInput")
    winit_d = nc.dram_tensor("winit", [K, BPC], f32, kind="ExternalInput")
    ones_d = nc.dram_tensor("ones", [1, K], f32, kind="ExternalInput")
    qout_d = nc.dram_tensor("qout", [G, NBLK, 128 * BG], f32,
                            kind="ExternalOutput")

    # QB steps of matmul output share one PSUM bank tile; q (row 0) is
    # copied to SBUF once per QB steps, and strips are DMA'd per 128 steps.
    QB = 8 if BG <= 64 else 4
    NS = 128  # steps per SBUF q-strip

    with tile.TileContext(nc) as tc:
        with (
            tc.tile_pool(name="const", bufs=1) as cpool,
            tc.tile_pool(name="ex", bufs=2) as expool,
            tc.tile_pool(name="w", bufs=3) as wpool,
            tc.tile_pool(name="rq", bufs=2) as spool,
            tc.tile_pool(name="qs", bufs=2) as qspool,
            tc.tile_pool(name="u", bufs=2, space="PSUM") as ppool,
            tc.tile_pool(name="bc", bufs=1, space="PSUM") as bpool,
        ):
            ehatT = cpool.tile([K, RROW + 1], f32)
            nc.sync.dma_start(ehatT[:], ehatT_d[:])
            ones = cpool.tile([1, K], f32)
            nc.sync.dma_start(ones[:], ones_d[:])

            W = []
            for g in range(G):
                w0 = wpool.tile([K, BG], f32, tag=f"w{g}", name=f"w0_{g}")
                nc.sync.dma_start(w0[:], winit_d[:, g * BG:(g + 1) * BG])
                W.append(w0)

            ex = None
            ua = [None] * G
            qsb = [None] * G
            for t in range(T + 1):
                if t % CH == 0 and t < T:
                    ex = expool.tile([K, CH, BPC], f32, tag="ex", name="ex")
                    nc.sync.dma_start(ex[:], expLT_d[:, t:t + CH, :])
                i = t % QB
                for g in range(G):
                    gs = slice(g * BG, (g + 1) * BG)
                    if t % NS == 0:
                        qsb[g] = qspool.tile([1, NS * BG], f32,
                                             tag=f"qs{g}", name=f"qs{g}")
                    if i == 0:
                        ua[g] = ppool.tile([RROW + 1, QB * BG], f32,
                                           tag=f"u{g}", name=f"u{g}")
                    u = ua[g]
                    # u[:, i] = [q_t ; E' @ W_t]  (one matmul per step)
                    nc.tensor.matmul(u[:, i * BG:(i + 1) * BG], ehatT[:],
                                     W[g][:], start=True, stop=True)
                    if t < T:
                        wn = wpool.tile([K, BG], f32, tag=f"w{g}",
                                        name=f"wn{g}")
                        nc.vector.tensor_mul(
                            wn[:], u[0:K, i * BG:(i + 1) * BG],
                            ex[:, t % CH, gs])
                        if t % RS == RS - 1:
                            rq = spool.tile([1, BG], f32, tag=f"rq{g}",
                                            name=f"rq{g}")
                            nc.vector.reciprocal(
                                rq[:], u[RROW:RROW + 1,
                                         i * BG:(i + 1) * BG])
                            bc = bpool.tile([K, BG], f32, tag=f"bc{g}",
                                            name=f"bc{g}")
                            nc.tensor.matmul(bc[:], ones[:], rq[:],
                                             start=True, stop=True)
                            wn2 = wpool.tile([K, BG], f32, tag=f"w{g}",
                                             name=f"wn2{g}")
                            nc.vector.tensor_mul(wn2[:], wn[:], bc[:])
                            wn = wn2
                        W[g] = wn
                    if i == QB - 1 or t == T:
                        # flush this group's q rows to the SBUF strip
                        n = (i + 1) * BG
                        off = (t - i) % NS * BG
                        nc.scalar.copy(qsb[g][0:1, off:off + n],
                                       u[RROW:RROW + 1, 0:n])
                    if t % NS == NS - 1 or t == T:
                        nc.sync.dma_start(
                            qout_d[g, t // NS, :], qsb[g][:])

    nc.compile()
    return nc


def _get_program(G):
    if G not in _PROG_CACHE:
        _PROG_CACHE[G] = _build_program(G)
    return _PROG_CACHE[G]


def _host_prep(logits, trans, G):
    Ep = np.exp(trans).astype(np.float32)                 # [K,K] E'[i,j]
    r = np.exp(trans[STOP]).astype(np.float32)            # [K]
    ehat = np.zeros((RROW + 1, K), np.float32)
    ehat[:K] = Ep
    ehat[RROW] = r
    ehatT = np.ascontiguousarray(ehat.T)                  # [K,RROW+1]
    ones = np.ones((1, K), np.float32)
    winit = np.zeros((K, BPC), np.float32)
    winit[START] = 1.0
    expL = np.exp(logits - C0)                            # [B,T,K] f32
    in_maps = []
    for c in range(NCORES):
        sh = expL[c * BPC:(c + 1) * BPC]                  # [128,T,K]
        expLT = np.ascontiguousarray(sh.transpose(2, 1, 0))  # [K,T,128]
        in_maps.append({"expLT": expLT, "ehatT": ehatT,
                        "winit": winit, "ones": ones})
    return in_maps


def _host_post(results, lens, G):
    """Reconstruct log-partition from the stored q stream."""
    BG = BPC // G
    partition = np.empty(B, np.float64)
    for c in range(NCORES):
        qout = results[c]["qout"]                         # [G,NBLK,128*BG]
        Qs = np.empty((T + 1, BPC), np.float64)
        for g in range(G):
            q = qout[g].reshape(NBLK * 128, BG)[:T + 1]   # [513,BG]
            Qs[:, g * BG:(g + 1) * BG] = q.astype(np.float64)
        lnQ = np.log(Qs[1:])                              # t=1..512 -> idx t-1
        # offset(t) = C0*t + sum_{t_r rescale, t_r < t} ln q_frame(t_r)
        offset = np.zeros((T + 1, BPC))
        for t_r in range(RS - 1, T, RS):                  # 31,63,...,511
            offset[t_r + 1:] += lnQ[t_r - 1]              # affects t > t_r
        tt = np.arange(1, T + 1)
        part_at = lnQ + C0 * tt[:, None] + offset[1:]
        lens_c = lens[c * BPC:(c + 1) * BPC]
        partition[c * BPC:(c + 1) * BPC] = \
            part_at[lens_c - 1, np.arange(BPC)]
    return partition


def _gold_scores(logits, trans, labels, lens):
    logits64 = logits.astype(np.float64)
    trans64 = trans.astype(np.float64)
    labels_ext = np.concatenate(
        [np.full((B, 1), START, np.int64), labels,
         np.full((B, 1), STOP, np.int64)], 1)
    pos = np.arange(T + 2)[None, :]
    labels_ext = np.where(pos < (lens + 1)[:, None], labels_ext, STOP)
    prev, nxt = labels_ext[:, :-1], labels_ext[:, 1:]
    m_trn = (np.arange(T + 1)[None, :] < (lens + 1)[:, None])
    transition_score = (trans64[nxt, prev] * m_trn).sum(1)
    em = np.take_along_axis(logits64, labels[:, :, None], 2)[:, :, 0]
    m_em = (np.arange(T)[None, :] < lens[:, None])
    emission_score = (em * m_em).sum(1)
    return emission_score, transition_score


def kernel(logits, transitions, labels, lens, _G=2, _trace=False):
    from concourse.bass_utils import run_bass_kernel_spmd

    logits = np.asarray(logits, dtype=np.float32)
    transitions = np.asarray(transitions, dtype=np.float32)
    labels_np = np.asarray(labels).astype(np.int64)
    lens_np = np.asarray(lens).astype(np.int64)

    nc = _get_program(_G)
    in_maps = _host_prep(logits, transitions, _G)
    out = run_bass_kernel_spmd(nc, in_maps, list(range(NCORES)),
                               trace=_trace)
    partition = _host_post(out.results, lens_np, _G)
    emission, transition = _gold_scores(logits, transitions, labels_np,
                                        lens_np)
    loss = partition + emission - transition
    if _trace:
        kernel._last_exec_ns = out.exec_time_ns
        kernel._last_profile = out.profile_json
    return loss.astype(np.float32)



# revision 4
# speedup vs baseline: 5.1179x; 5.1179x over previous
"""CRF negative-log-likelihood loss on 8 Trainium2 NeuronCores.

Data-parallel over batch (128 sequences/core) + segmented time axis.

The forward (log-partition) recurrence runs on device in the exp domain:

    W_{t+1} = (E' @ W_t) * exp(logits_t - C0),   E' = exp(transitions)

Because products of positive matrices contract to rank-1 (Perron-Frobenius),
the true state *direction* at a segment boundary can be recovered by running
the recurrence over just the last LA steps of the segment from an arbitrary
positive start.  This breaks the length-512 serial chain into NSEG=16
independent segments:

  phase A (LA=8 steps):  per segment, evolve an all-ones start over the
      segment's last 8 steps -> boundary direction vectors.
  phase B (L=32 steps):  all 16 segments evolve in parallel from those
      directions (segment 0 from the true start), writing bf16 W snapshots
      to HBM every J=8 steps.

Per core the 16 segments x 128 batch = 2048 chains are packed as 2 streams
x [128 partitions (2 vertical bands of 52 tags), 512 columns]; each stream
step is 2 concurrent quadrant matmuls (tile_position packing) + 1 DVE
multiply.  The host reconstructs log Z at t=lens exactly in f64 from the
snapshots, stitching per-segment scales by telescoped ratios; gold-path
emission/transition scores are cheap host-side gathers.
"""

import numpy as np

# problem constants (hardcoded per contract)
B, T, K = 1024, 512, 52
START, STOP = 50, 51
NCORES = 8
BPC = B // NCORES          # 128 sequences per core
C0 = 5.0                   # per-step log-shift folded into exp(logits)
L = 32                     # segment length
LA = 8                     # phase A (direction bootstrap) steps
J = 8                      # snapshot interval in phase B
NSEG = T // L              # 16 segments
NST = 2                    # streams (independent instruction chains)
CW = 512                   # columns per stream
NSNAP = L // J             # 4 snapshots per segment

_PROG_CACHE = {}


def _build_program():
    import concourse.mybir as mybir
    import concourse.tile as tile
    from concourse import bacc

    f32 = mybir.dt.float32
    bf16 = mybir.dt.bfloat16

    nc = bacc.Bacc("TRN2", target_bir_lowering=False, debug=False,
                   num_devices=NCORES)
    # e[st, p, l, c]: exp(logits - C0) for the chain at (stream st, col c),
    # tag p%64, band p//64, local step l.  Rows 52-63 / 116-127 are zero.
    e_d = nc.dram_tensor("e", [NST, 128, L, CW], bf16, kind="ExternalInput")
    # ehat[j, i] = E'[i, j] (plus zero cols 52-63), replicated at rows 64+.
    ehat_d = nc.dram_tensor("ehat", [128, 64], bf16, kind="ExternalInput")
    w0_d = nc.dram_tensor("w0", [64, 128], bf16, kind="ExternalInput")
    snapA_d = nc.dram_tensor("snapA", [NST, 128, CW], bf16,
                             kind="ExternalOutput")
    snapB_d = nc.dram_tensor("snapB", [NST, NSNAP, 128, CW], bf16,
                             kind="ExternalOutput")

    with tile.TileContext(nc) as tc:
        with (
            tc.tile_pool(name="const", bufs=1) as cpool,
            tc.tile_pool(name="e3", bufs=1) as e3pool,
            tc.tile_pool(name="ech", bufs=3) as epool,
            tc.tile_pool(name="w", bufs=8) as wpool,
            tc.tile_pool(name="u", bufs=3, space="PSUM") as upool,
        ):
            ehat = cpool.tile([128, 64], bf16, tag="ehat")
            nc.sync.dma_start(ehat[:], ehat_d[:])
            w0t = cpool.tile([64, 128], bf16, tag="w0t")
            nc.sync.dma_start(w0t[:], w0_d[:])

            # last 8-step chunk first (phase A runs on it), then chunks 0-2
            e3 = []
            for st in range(NST):
                t_ = e3pool.tile([128, J, CW], bf16, tag=f"e3_{st}",
                                 name=f"e3_{st}")
                nc.sync.dma_start(t_[:], e_d[st, :, L - J:L, :])
                e3.append(t_)
            ech = [[None] * 3 for _ in range(NST)]
            for ch in range(3):
                for st in range(NST):
                    t_ = epool.tile([128, J, CW], bf16, tag=f"e{st}",
                                    name=f"e{st}_{ch}")
                    nc.sync.dma_start(t_[:], e_d[st, :, J * ch:J * ch + J, :])
                    ech[st][ch] = t_

            def step(W, st, et):
                """One recurrence step for stream st: 2 quadrant matmuls
                (concurrent in the PE array) + 1 elementwise multiply."""
                u = upool.tile([128, CW], f32, tag=f"u{st}", name=f"u{st}")
                nc.tensor.matmul(u[0:64, :], ehat[0:52, :], W[0:52, :],
                                 start=True, stop=True)
                nc.tensor.matmul(u[64:128, :], ehat[64:116, :],
                                 W[64:116, :], start=True, stop=True)
                wn = wpool.tile([128, CW], bf16, tag=f"w{st}",
                                name=f"wn{st}")
                nc.vector.tensor_mul(wn[:], u[:], et)
                return wn

            # ---- phase A: bootstrap boundary directions from ones ----
            W = []
            for st in range(NST):
                wa = wpool.tile([128, CW], bf16, tag=f"w{st}",
                                name=f"wa{st}")
                nc.vector.memset(wa[:], 1.0)
                W.append(wa)
            for el in range(LA):
                for st in range(NST):
                    W[st] = step(W[st], st, e3[st][:, el, :])
            for st in range(NST):
                nc.scalar.dma_start(snapA_d[st], W[st][:])

            # ---- boundary: W_B(seg s+1) = W_A(seg s); seg 0 = true w0 ----
            # segment s lives at stream s//8, band (s%8)//4, col block s%4.
            Wb = []
            for st in range(NST):
                wb = wpool.tile([128, CW], bf16, tag=f"w{st}",
                                name=f"wb{st}")
                # same (stream, band): col block shift by 128
                nc.scalar.copy(wb[:, 128:CW], W[st][:, 0:CW - 128])
                Wb.append(wb)
            # cross band/stream shifts need a partition move -> SBUF DMA
            nc.scalar.dma_start(Wb[0][64:128, 0:128], W[0][0:64, 384:CW])
            nc.scalar.dma_start(Wb[1][0:64, 0:128], W[0][64:128, 384:CW])
            nc.scalar.dma_start(Wb[1][64:128, 0:128], W[1][0:64, 384:CW])
            nc.scalar.dma_start(Wb[0][0:64, 0:128], w0t[:])
            W = Wb

            # ---- phase B: all segments in parallel, snapshot every J ----
            for el in range(L):
                for st in range(NST):
                    et = (ech[st][el // J][:, el % J, :] if el < L - J
                          else e3[st][:, el % J, :])
                    W[st] = step(W[st], st, et)
                if (el + 1) % J == 0:
                    for st in range(NST):
                        nc.scalar.dma_start(
                            snapB_d[st, (el + 1) // J - 1], W[st][:])

    nc.compile()
    return nc


def _get_program():
    if "p" not in _PROG_CACHE:
        _PROG_CACHE["p"] = _build_program()
    return _PROG_CACHE["p"]


def _to_bf16_np(x):
    import ml_dtypes
    return np.asarray(x, np.float32).astype(ml_dtypes.bfloat16)


def _host_prep(logits, trans):
    """Build per-core input maps."""
    Ep = np.exp(trans.astype(np.float64)).astype(np.float32)    # [K,K]
    ehat = np.zeros((128, 64), np.float32)
    ehat[0:K, 0:K] = Ep.T                                       # [j, i]
    ehat[64:64 + K, 0:K] = Ep.T
    w0 = np.zeros((64, 128), np.float32)
    w0[START, :] = 1.0

    ex = np.exp(logits.astype(np.float32) - C0)                 # [B,T,K]
    # [core, b, seg, l, k]
    ex_r = ex.reshape(NCORES, BPC, NSEG, L, K)
    eD = np.zeros((NCORES, NST, 128, L, CW), np.float32)
    for s in range(NSEG):
        st, v, blk = s // 8, (s % 8) // 4, s % 4
        # [core, k, l, b]
        eD[:, st, 64 * v:64 * v + K, :, 128 * blk:128 * blk + 128] = \
            ex_r[:, :, s, :, :].transpose(0, 3, 2, 1)

    ehat_b = _to_bf16_np(ehat)
    w0_b = _to_bf16_np(w0)
    in_maps = []
    for c in range(NCORES):
        in_maps.append({"e": _to_bf16_np(eD[c]), "ehat": ehat_b,
                        "w0": w0_b})
    return in_maps


def _host_post(results, logits, trans, lens):
    """Reconstruct log Z at t=lens per sequence, in f64, from snapshots."""
    Ep64 = np.exp(trans.astype(np.float64))                     # [K,K]
    r64 = Ep64[STOP]                                            # [K]
    logits64 = logits.astype(np.float64)

    # y[s, k, b, :] = state of segment s after k*J steps (k=0..NSNAP)
    y = np.zeros((NSEG, NSNAP + 1, B, K))
    for c in range(NCORES):
        snapA = np.asarray(results[c]["snapA"], np.float32)     # [2,128,CW]
        snapB = np.asarray(results[c]["snapB"], np.float32)     # [2,4,128,CW]
        bs = slice(c * BPC, (c + 1) * BPC)
        for s in range(NSEG):
            st, v, blk = s // 8, (s % 8) // 4, s % 4
            rows = slice(64 * v, 64 * v + K)
            cols = slice(128 * blk, 128 * blk + 128)
            # y^s_0: segment 0 = true start; else phase A end of segment s-1
            if s == 0:
                y[0, 0, bs, START] = 1.0
            for k in range(NSNAP):
                y[s, k + 1, bs] = snapB[st, k, rows, cols].T
            if s < NSEG - 1:
                # phase A end of segment s (at segment s's own position)
                # is the device's B-init for segment s+1
                y[s + 1, 0, bs] = snapA[st, rows, cols].T

    # telescoped segment scales
    lnc = np.zeros((NSEG, B))
    for s in range(1, NSEG):
        num = y[s - 1, NSNAP] @ r64
        den = y[s, 0] @ r64
        lnc[s] = lnc[s - 1] + np.log(num) - np.log(den)

    t_all = lens.astype(np.int64)                               # [B], 1..512
    s_all = (t_all - 1) // L
    lpos = t_all - s_all * L                                    # 1..L
    k_all = lpos // J                                           # 0..NSNAP
    steps = lpos - k_all * J                                    # 0..J-1
    t0_all = s_all * L + k_all * J

    Wf = y[s_all, k_all, np.arange(B)]                          # [B, K]
    e64 = None
    for n in range(1, J):
        sel = steps >= n
        if not np.any(sel):
            continue
        if e64 is None:
            e64 = np.exp(logits64 - C0)
        tt = t0_all[sel] + n - 1
        Wf[sel] = (Wf[sel] @ Ep64.T) * e64[sel, tt, :]
    part = (np.log(Wf @ r64) + lnc[s_all, np.arange(B)]
            + C0 * t_all)
    return part


def _gold_scores(logits, trans, labels, lens):
    logits64 = logits.astype(np.float64)
    trans64 = trans.astype(np.float64)
    labels_ext = np.concatenate(
        [np.full((B, 1), START, np.int64), labels,
         np.full((B, 1), STOP, np.int64)], 1)
    pos = np.arange(T + 2)[None, :]
    labels_ext = np.where(pos < (lens + 1)[:, None], labels_ext, STOP)
    prev, nxt = labels_ext[:, :-1], labels_ext[:, 1:]
    m_trn = (np.arange(T + 1)[None, :] < (lens + 1)[:, None])
    transition_score = (trans64[nxt, prev] * m_trn).sum(1)
    em = np.take_along_axis(logits64, labels[:, :, None], 2)[:, :, 0]
    m_em = (np.arange(T)[None, :] < lens[:, None])
    emission_score = (em * m_em).sum(1)
    return emission_score, transition_score


def kernel(logits, transitions, labels, lens, _trace=False):
    from concourse.bass_utils import run_bass_kernel_spmd

    logits = np.asarray(logits, dtype=np.float32)
    transitions = np.asarray(transitions, dtype=np.float32)
    labels_np = np.asarray(labels).astype(np.int64)
    lens_np = np.asarray(lens).astype(np.int64)

    nc = _get_program()
    in_maps = _host_prep(logits, transitions)
    out = run_bass_kernel_spmd(nc, in_maps, list(range(NCORES)),
                               trace=_trace)
    partition = _host_post(out.results, logits, transitions, lens_np)
    emission, transition = _gold_scores(logits, transitions, labels_np,
                                        lens_np)
    loss = partition + emission - transition
    if _trace:
        kernel._last_exec_ns = out.exec_time_ns
        kernel._last_profile = out.profile_json
    return loss.astype(np.float32)


# revision 5
# speedup vs baseline: 5.5238x; 1.0793x over previous
"""CRF negative-log-likelihood loss on 8 Trainium2 NeuronCores.

Data-parallel over batch (128 sequences/core) + segmented time axis.

The forward (log-partition) recurrence runs on device in the exp domain:

    W_{t+1} = (E' @ W_t) * exp(logits_t - C0),   E' = exp(transitions)

Because products of positive matrices contract to rank-1 (Perron-Frobenius),
the true state *direction* at a segment boundary can be recovered by running
the recurrence over just the last LA steps of the segment from an arbitrary
positive start.  This breaks the length-512 serial chain into NSEG
independent segments:

  phase A (LA steps):  per segment, evolve an all-ones start over the
      segment's last LA steps -> boundary direction vectors.
  phase B (L steps):   all segments evolve in parallel from those
      directions (segment 0 from the true start), writing bf16 W snapshots
      to HBM every J steps.

Per core the NSEG x 128 batch chains are packed as NST streams x
[128 partitions (2 vertical bands of 52 tags), 512 columns]; each stream
step is 2 concurrent quadrant matmuls (tile_position packing) + 1 DVE
multiply.  The host reconstructs log Z at t=lens exactly in f64 from the
snapshots, stitching per-segment scales by telescoped ratios; gold-path
emission/transition scores are cheap host-side gathers.
"""

import numpy as np

# problem constants (hardcoded per contract)
B, T, K = 1024, 512, 52
START, STOP = 50, 51
NCORES = 8
BPC = B // NCORES          # 128 sequences per core
C0 = 5.0                   # per-step log-shift folded into exp(logits)
L = 16                     # segment length
LA = 4                     # phase A (direction bootstrap) steps
J = 8                      # snapshot interval in phase B
NSEG = T // L              # 32 segments
NST = 4                    # streams (independent instruction chains)
CW = 512                   # columns per stream
NSNAP = L // J             # snapshots per segment
NWARM = 40                 # PE warm-up matmuls (HAM un-throttle)

_PROG_CACHE = {}


def _build_program():
    import concourse.mybir as mybir
    import concourse.tile as tile
    from concourse import bacc

    f32 = mybir.dt.float32
    bf16 = mybir.dt.bfloat16

    nc = bacc.Bacc("TRN2", target_bir_lowering=False, debug=False,
                   num_devices=NCORES)
    # e[st, p, l, c]: exp(logits - C0) for the chain at (stream st, col c),
    # tag p%64, band p//64, local step l.  Rows 52-63 / 116-127 are zero.
    e_d = nc.dram_tensor("e", [NST, 128, L, CW], bf16, kind="ExternalInput")
    # ehat[j, i] = E'[i, j] (plus zero cols 52-63), replicated at rows 64+.
    ehat_d = nc.dram_tensor("ehat", [128, 64], bf16, kind="ExternalInput")
    w0_d = nc.dram_tensor("w0", [64, 128], bf16, kind="ExternalInput")
    snapA_d = nc.dram_tensor("snapA", [NST, 128, CW], bf16,
                             kind="ExternalOutput")
    snapB_d = nc.dram_tensor("snapB", [NST, NSNAP, 128, CW], bf16,
                             kind="ExternalOutput")

    with tile.TileContext(nc) as tc:
        with (
            tc.tile_pool(name="const", bufs=1) as cpool,
            tc.tile_pool(name="e3", bufs=1) as e3pool,
            tc.tile_pool(name="ech", bufs=1) as epool,
            tc.tile_pool(name="w", bufs=6) as wpool,
            tc.tile_pool(name="u", bufs=1, space="PSUM") as upool,
            tc.tile_pool(name="wu", bufs=1, space="PSUM") as wupool,
        ):
            ehat = cpool.tile([128, 64], bf16, tag="ehat")
            nc.sync.dma_start(ehat[:], ehat_d[:])
            w0t = cpool.tile([64, 128], bf16, tag="w0t")
            nc.sync.dma_start(w0t[:], w0_d[:])

            # phase-A chunk (slices 8-15) first, then phase-B chunk 0-7
            e3 = []
            for st in range(NST):
                t_ = e3pool.tile([128, J, CW], bf16, tag=f"e3_{st}",
                                 name=f"e3_{st}")
                nc.sync.dma_start(t_[:], e_d[st, :, L - J:L, :])
                e3.append(t_)
            ech = []
            for st in range(NST):
                t_ = epool.tile([128, J, CW], bf16, tag=f"e0_{st}",
                                name=f"e0_{st}")
                nc.sync.dma_start(t_[:], e_d[st, :, 0:J, :])
                ech.append(t_)

            # PE warm-up: dense dummy matmuls while the e DMAs stream in,
            # so HAM un-throttles the PE clock before the chains start.
            for i in range(NWARM):
                wu = wupool.tile([64, 64], f32, tag="wu", name="wu")
                nc.tensor.matmul(wu[:], ehat[0:52, 0:64], ehat[0:52, 0:64],
                                 start=True, stop=True)

            def step(W, st, et):
                """One recurrence step for stream st: 2 quadrant matmuls
                (concurrent in the PE array) + 1 elementwise multiply."""
                u = upool.tile([128, CW], f32, tag=f"u{st}", name=f"u{st}")
                nc.tensor.matmul(u[0:64, :], ehat[0:52, :], W[0:52, :],
                                 start=True, stop=True)
                nc.tensor.matmul(u[64:128, :], ehat[64:116, :],
                                 W[64:116, :], start=True, stop=True)
                wn = wpool.tile([128, CW], bf16, tag=f"w{st}",
                                name=f"wn{st}")
                nc.vector.tensor_mul(wn[:], u[:], et)
                return wn

            # ---- phase A: bootstrap boundary directions from ones ----
            W = []
            for st in range(NST):
                wa = wpool.tile([128, CW], bf16, tag=f"w{st}",
                                name=f"wa{st}")
                nc.vector.memset(wa[:], 1.0)
                W.append(wa)
            for el in range(LA):
                for st in range(NST):
                    W[st] = step(W[st], st, e3[st][:, (J - LA) + el, :])
            for st in range(NST):
                nc.scalar.dma_start(snapA_d[st], W[st][:])

            # ---- boundary: W_B(seg s+1) = W_A(seg s); seg 0 = true w0 ----
            # segment s lives at stream s//8, band (s%8)//4, col block s%4.
            Wb = []
            for st in range(NST):
                wb = wpool.tile([128, CW], bf16, tag=f"w{st}",
                                name=f"wb{st}")
                # same (stream, band): col block shift by 128
                nc.scalar.copy(wb[:, 128:CW], W[st][:, 0:CW - 128])
                Wb.append(wb)
            # cross band/stream shifts need a partition move -> SBUF DMA
            for st in range(NST):
                nc.scalar.dma_start(Wb[st][64:128, 0:128],
                                    W[st][0:64, CW - 128:CW])
            for st in range(NST - 1):
                nc.scalar.dma_start(Wb[st + 1][0:64, 0:128],
                                    W[st][64:128, CW - 128:CW])
            nc.scalar.dma_start(Wb[0][0:64, 0:128], w0t[:])
            W = Wb

            # ---- phase B: all segments in parallel, snapshot every J ----
            for el in range(L):
                for st in range(NST):
                    et = (ech[st][:, el, :] if el < J
                          else e3[st][:, el - J, :])
                    W[st] = step(W[st], st, et)
                if (el + 1) % J == 0:
                    for st in range(NST):
                        nc.scalar.dma_start(
                            snapB_d[st, (el + 1) // J - 1], W[st][:])

    nc.compile()
    return nc


def _get_program():
    if "p" not in _PROG_CACHE:
        _PROG_CACHE["p"] = _build_program()
    return _PROG_CACHE["p"]


def _to_bf16_np(x):
    import ml_dtypes
    return np.asarray(x, np.float32).astype(ml_dtypes.bfloat16)


def _host_prep(logits, trans):
    """Build per-core input maps."""
    Ep = np.exp(trans.astype(np.float64)).astype(np.float32)    # [K,K]
    ehat = np.zeros((128, 64), np.float32)
    ehat[0:K, 0:K] = Ep.T                                       # [j, i]
    ehat[64:64 + K, 0:K] = Ep.T
    w0 = np.zeros((64, 128), np.float32)
    w0[START, :] = 1.0

    ex = np.exp(logits.astype(np.float32) - C0)                 # [B,T,K]
    # [core, b, seg, l, k]
    ex_r = ex.reshape(NCORES, BPC, NSEG, L, K)
    eD = np.zeros((NCORES, NST, 128, L, CW), np.float32)
    for s in range(NSEG):
        st, v, blk = s // 8, (s % 8) // 4, s % 4
        # [core, k, l, b]
        eD[:, st, 64 * v:64 * v + K, :, 128 * blk:128 * blk + 128] = \
            ex_r[:, :, s, :, :].transpose(0, 3, 2, 1)

    ehat_b = _to_bf16_np(ehat)
    w0_b = _to_bf16_np(w0)
    in_maps = []
    for c in range(NCORES):
        in_maps.append({"e": _to_bf16_np(eD[c]), "ehat": ehat_b,
                        "w0": w0_b})
    return in_maps


def _host_post(results, logits, trans, lens):
    """Reconstruct log Z at t=lens per sequence, in f64, from snapshots."""
    Ep64 = np.exp(trans.astype(np.float64))                     # [K,K]
    r64 = Ep64[STOP]                                            # [K]
    logits64 = logits.astype(np.float64)

    # y[s, k, b, :] = state of segment s after k*J steps (k=0..NSNAP)
    y = np.zeros((NSEG, NSNAP + 1, B, K))
    for c in range(NCORES):
        snapA = np.asarray(results[c]["snapA"], np.float32)
        snapB = np.asarray(results[c]["snapB"], np.float32)
        bs = slice(c * BPC, (c + 1) * BPC)
        for s in range(NSEG):
            st, v, blk = s // 8, (s % 8) // 4, s % 4
            rows = slice(64 * v, 64 * v + K)
            cols = slice(128 * blk, 128 * blk + 128)
            if s == 0:
                y[0, 0, bs, START] = 1.0
            for k in range(NSNAP):
                y[s, k + 1, bs] = snapB[st, k, rows, cols].T
            if s < NSEG - 1:
                # phase A end of segment s (at segment s's own position)
                # is the device's B-init for segment s+1
                y[s + 1, 0, bs] = snapA[st, rows, cols].T

    # telescoped segment scales
    lnc = np.zeros((NSEG, B))
    for s in range(1, NSEG):
        num = y[s - 1, NSNAP] @ r64
        den = y[s, 0] @ r64
        lnc[s] = lnc[s - 1] + np.log(num) - np.log(den)

    t_all = lens.astype(np.int64)                               # [B], 1..512
    s_all = (t_all - 1) // L
    lpos = t_all - s_all * L                                    # 1..L
    k_all = lpos // J                                           # 0..NSNAP
    steps = lpos - k_all * J                                    # 0..J-1
    t0_all = s_all * L + k_all * J

    Wf = y[s_all, k_all, np.arange(B)]                          # [B, K]
    e64 = None
    for n in range(1, J):
        sel = steps >= n
        if not np.any(sel):
            continue
        if e64 is None:
            e64 = np.exp(logits64 - C0)
        tt = t0_all[sel] + n - 1
        Wf[sel] = (Wf[sel] @ Ep64.T) * e64[sel, tt, :]
    part = (np.log(Wf @ r64) + lnc[s_all, np.arange(B)]
            + C0 * t_all)
    return part


def _gold_scores(logits, trans, labels, lens):
    logits64 = logits.astype(np.float64)
    trans64 = trans.astype(np.float64)
    labels_ext = np.concatenate(
        [np.full((B, 1), START, np.int64), labels,
         np.full((B, 1), STOP, np.int64)], 1)
    pos = np.arange(T + 2)[None, :]
    labels_ext = np.where(pos < (lens + 1)[:, None], labels_ext, STOP)
    prev, nxt = labels_ext[:, :-1], labels_ext[:, 1:]
    m_trn = (np.arange(T + 1)[None, :] < (lens + 1)[:, None])
    transition_score = (trans64[nxt, prev] * m_trn).sum(1)
    em = np.take_along_axis(logits64, labels[:, :, None], 2)[:, :, 0]
    m_em = (np.arange(T)[None, :] < lens[:, None])
    emission_score = (em * m_em).sum(1)
    return emission_score, transition_score


def kernel(logits, transitions, labels, lens, _trace=False):
    from concourse.bass_utils import run_bass_kernel_spmd

    logits = np.asarray(logits, dtype=np.float32)
    transitions = np.asarray(transitions, dtype=np.float32)
    labels_np = np.asarray(labels).astype(np.int64)
    lens_np = np.asarray(lens).astype(np.int64)

    nc = _get_program()
    in_maps = _host_prep(logits, transitions)
    out = run_bass_kernel_spmd(nc, in_maps, list(range(NCORES)),
                               trace=_trace)
    partition = _host_post(out.results, logits, transitions, lens_np)
    emission, transition = _gold_scores(logits, transitions, labels_np,
                                        lens_np)
    loss = partition + emission - transition
    if _trace:
        kernel._last_exec_ns = out.exec_time_ns
        kernel._last_profile = out.profile_json
    return loss.astype(np.float32)


# revision 7
# speedup vs baseline: 6.1932x; 1.1212x over previous
"""CRF negative-log-likelihood loss on 8 Trainium2 NeuronCores.

Data-parallel over batch (128 sequences/core) + segmented time axis with
overlapped warm-up.

The forward (log-partition) recurrence runs on device in the exp domain:

    W_{t+1} = (E' @ W_t) * exp(logits_t - C0),   E' = exp(transitions)

Products of positive matrices contract to rank-1 (Perron-Frobenius), so the
state *direction* at any t is recovered by running the recurrence over the
preceding OV steps from an arbitrary positive start.  Each length-L segment
therefore runs OV warm-up steps (over the previous segment's last OV inputs,
from an all-ones start) followed by its own L steps -- all NSEG segments in
parallel, no cross-segment communication.  Segment 0's warm-up inputs are a
START-tag indicator, which holds its state exactly proportional to the true
start vector.  bf16 W snapshots go to HBM every J steps; the host
reconstructs log Z at t=lens in f64 from the snapshot preceding it,
stitching per-segment scales by telescoped ratios anchored at the true w0.
Gold-path emission/transition scores are host-side gathers.

Per core the NSEG x 128 batch chains pack as NST streams x [128 partitions
(2 vertical bands of 52 tags), 512 columns]; each stream step is 2
concurrent quadrant matmuls (tile_position packing) + 1 DVE multiply.
"""

import numpy as np

# problem constants (hardcoded per contract)
B, T, K = 1024, 512, 52
START, STOP = 50, 51
NCORES = 8
BPC = B // NCORES          # 128 sequences per core
C0 = 5.0                   # per-step log-shift folded into exp(logits)
L = 16                     # segment length
OV = 4                     # warm-up (direction bootstrap) steps per segment
J = 8                      # snapshot interval
NSEG = T // L              # 32 segments
NST = 4                    # streams (independent instruction chains)
CW = 512                   # columns per stream
NSTEP = L + OV             # 20 device steps per chain
NSNAP = L // J + 1         # 3 snapshots (at local t-offsets 0, J, 2J)
NWARM = 24                 # PE warm-up matmuls (HAM un-throttle)

_PROG_CACHE = {}


def _build_program():
    import concourse.mybir as mybir
    import concourse.tile as tile
    from concourse import bacc

    f32 = mybir.dt.float32
    bf16 = mybir.dt.bfloat16

    nc = bacc.Bacc("TRN2", target_bir_lowering=False, debug=False,
                   num_devices=NCORES)
    # e[st, p, l, c]: step multiplier for the chain at (stream st, col c),
    # tag p%64, band p//64, local step l (l<OV: warm-up slices).
    e_d = nc.dram_tensor("e", [NST, 128, NSTEP, CW], bf16,
                         kind="ExternalInput")
    # ehat[j, i] = E'[i, j] (plus zero cols 52-63), replicated at rows 64+.
    ehat_d = nc.dram_tensor("ehat", [128, 64], bf16, kind="ExternalInput")
    snap_d = nc.dram_tensor("snap", [NST, NSNAP, 128, CW], bf16,
                            kind="ExternalOutput")

    # e chunks per stream: [0:8), [8:16), [16:20)
    CH = [(0, J), (J, 2 * J), (2 * J, NSTEP)]

    with tile.TileContext(nc) as tc:
        with (
            tc.tile_pool(name="const", bufs=1) as cpool,
            tc.tile_pool(name="ech", bufs=1) as epool,
            tc.tile_pool(name="w", bufs=6) as wpool,
            tc.tile_pool(name="u", bufs=1, space="PSUM") as upool,
            tc.tile_pool(name="wu", bufs=1, space="PSUM") as wupool,
        ):
            ehat = cpool.tile([128, 64], bf16, tag="ehat")
            nc.sync.dma_start(ehat[:], ehat_d[:])

            # input chunks split across both HWDGE rings (SP: even streams,
            # ACT: odd) and ordered by first use
            ech = [[None] * len(CH) for _ in range(NST)]
            for ci, (c0, c1) in enumerate(CH):
                for st in range(NST):
                    t_ = epool.tile([128, c1 - c0, CW], bf16,
                                    tag=f"e{st}_{ci}", name=f"e{st}_{ci}")
                    eng = nc.sync if st % 2 == 0 else nc.scalar
                    eng.dma_start(t_[:], e_d[st, :, c0:c1, :])
                    ech[st][ci] = t_

            # PE warm-up: dense dummy matmuls while the e DMAs stream in,
            # so HAM un-throttles the PE clock before the chains start.
            for i in range(NWARM):
                wu = wupool.tile([64, 64], f32, tag="wu", name="wu")
                nc.tensor.matmul(wu[:], ehat[0:52, 0:64], ehat[0:52, 0:64],
                                 start=True, stop=True)

            # all chains start from ones; segment 0's warm-up input mask
            # collapses its state onto the true start direction
            W = []
            for st in range(NST):
                wa = wpool.tile([128, CW], bf16, tag=f"w{st}",
                                name=f"wa{st}")
                nc.gpsimd.memset(wa[:], 1.0)
                W.append(wa)

            for el in range(NSTEP):
                ci = min(el // J, 2)
                for st in range(NST):
                    et = ech[st][ci][:, el - CH[ci][0], :]
                    u = upool.tile([128, CW], f32, tag=f"u{st}",
                                   name=f"u{st}")
                    nc.tensor.matmul(u[0:64, :], ehat[0:52, :],
                                     W[st][0:52, :], start=True, stop=True)
                    nc.tensor.matmul(u[64:128, :], ehat[64:116, :],
                                     W[st][64:116, :], start=True, stop=True)
                    wn = wpool.tile([128, CW], bf16, tag=f"w{st}",
                                    name=f"wn{st}")
                    nc.vector.tensor_mul(wn[:], u[:], et)
                    W[st] = wn
                off = el + 1 - OV        # real t-offset of the new state
                if off >= 0 and off % J == 0:
                    for st in range(NST):
                        eng = nc.scalar if st % 2 == 0 else nc.sync
                        eng.dma_start(snap_d[st, off // J], W[st][:])

    nc.compile()
    return nc


def _get_program():
    if "p" not in _PROG_CACHE:
        _PROG_CACHE["p"] = _build_program()
    return _PROG_CACHE["p"]


def _to_bf16_np(x):
    import ml_dtypes
    return np.asarray(x, np.float32).astype(ml_dtypes.bfloat16)


def _host_prep(logits, trans):
    """Build per-core input maps."""
    Ep = np.exp(trans.astype(np.float64)).astype(np.float32)    # [K,K]
    ehat = np.zeros((128, 64), np.float32)
    ehat[0:K, 0:K] = Ep.T                                       # [j, i]
    ehat[64:64 + K, 0:K] = Ep.T

    ex = np.exp(logits.astype(np.float32) - C0)                 # [B,T,K]
    # [core, b, seg, l, k]
    ex_r = ex.reshape(NCORES, BPC, NSEG, L, K)
    eD = np.zeros((NCORES, NST, 128, NSTEP, CW), np.float32)
    for s in range(NSEG):
        st, v, blk = s // 8, (s % 8) // 4, s % 4
        rows = slice(64 * v, 64 * v + K)
        cols = slice(128 * blk, 128 * blk + 128)
        # [core, k, l, b]
        eD[:, st, rows, OV:, cols] = \
            ex_r[:, :, s, :, :].transpose(0, 3, 2, 1)
        if s == 0:
            # START indicator holds the state on the true start direction
            eD[:, st, 64 * v + START, 0:OV, cols] = 1.0
        else:
            eD[:, st, rows, 0:OV, cols] = \
                ex_r[:, :, s - 1, L - OV:, :].transpose(0, 3, 2, 1)

    ehat_b = _to_bf16_np(ehat)
    in_maps = []
    for c in range(NCORES):
        in_maps.append({"e": _to_bf16_np(eD[c]), "ehat": ehat_b})
    return in_maps


def _host_post(results, logits, trans, lens):
    """Reconstruct log Z at t=lens per sequence, in f64, from snapshots."""
    Ep64 = np.exp(trans.astype(np.float64))                     # [K,K]
    r64 = Ep64[STOP]                                            # [K]
    logits64 = logits.astype(np.float64)

    # y[s, k, b, :] = device state of segment s at absolute t = s*L + k*J
    y = np.zeros((NSEG, NSNAP, B, K))
    for c in range(NCORES):
        snap = np.asarray(results[c]["snap"], np.float32)
        bs = slice(c * BPC, (c + 1) * BPC)
        for s in range(NSEG):
            st, v, blk = s // 8, (s % 8) // 4, s % 4
            rows = slice(64 * v, 64 * v + K)
            cols = slice(128 * blk, 128 * blk + 128)
            for k in range(NSNAP):
                y[s, k, bs] = snap[st, k, rows, cols].T

    # telescoped segment scales, anchored at the exact start vector w0
    # (r.w0 = r[START]): W_true(s*L) = gamma_s * y[s, 0]
    lnc = np.zeros((NSEG, B))
    lnc[0] = np.log(r64[START]) - np.log(y[0, 0] @ r64)
    for s in range(1, NSEG):
        num = y[s - 1, NSNAP - 1] @ r64
        den = y[s, 0] @ r64
        lnc[s] = lnc[s - 1] + np.log(num) - np.log(den)

    t_all = lens.astype(np.int64)                               # [B], 1..512
    s_all = (t_all - 1) // L
    lpos = t_all - s_all * L                                    # 1..L
    k_all = lpos // J                                           # 0..2
    steps = lpos - k_all * J                                    # 0..J-1
    t0_all = s_all * L + k_all * J

    Wf = y[s_all, k_all, np.arange(B)]                          # [B, K]
    e64 = None
    for n in range(1, J):
        sel = steps >= n
        if not np.any(sel):
            continue
        if e64 is None:
            e64 = np.exp(logits64 - C0)
        tt = t0_all[sel] + n - 1
        Wf[sel] = (Wf[sel] @ Ep64.T) * e64[sel, tt, :]
    part = (np.log(Wf @ r64) + lnc[s_all, np.arange(B)]
            + C0 * t_all)
    return part


def _gold_scores(logits, trans, labels, lens):
    logits64 = logits.astype(np.float64)
    trans64 = trans.astype(np.float64)
    labels_ext = np.concatenate(
        [np.full((B, 1), START, np.int64), labels,
         np.full((B, 1), STOP, np.int64)], 1)
    pos = np.arange(T + 2)[None, :]
    labels_ext = np.where(pos < (lens + 1)[:, None], labels_ext, STOP)
    prev, nxt = labels_ext[:, :-1], labels_ext[:, 1:]
    m_trn = (np.arange(T + 1)[None, :] < (lens + 1)[:, None])
    transition_score = (trans64[nxt, prev] * m_trn).sum(1)
    em = np.take_along_axis(logits64, labels[:, :, None], 2)[:, :, 0]
    m_em = (np.arange(T)[None, :] < lens[:, None])
    emission_score = (em * m_em).sum(1)
    return emission_score, transition_score


def kernel(logits, transitions, labels, lens, _trace=False):
    from concourse.bass_utils import run_bass_kernel_spmd

    logits = np.asarray(logits, dtype=np.float32)
    transitions = np.asarray(transitions, dtype=np.float32)
    labels_np = np.asarray(labels).astype(np.int64)
    lens_np = np.asarray(lens).astype(np.int64)

    nc = _get_program()
    in_maps = _host_prep(logits, transitions)
    out = run_bass_kernel_spmd(nc, in_maps, list(range(NCORES)),
                               trace=_trace)
    partition = _host_post(out.results, logits, transitions, lens_np)
    emission, transition = _gold_scores(logits, transitions, labels_np,
                                        lens_np)
    loss = partition + emission - transition
    if _trace:
        kernel._last_exec_ns = out.exec_time_ns
        kernel._last_profile = out.profile_json
    return loss.astype(np.float32)


# revision 8
# speedup vs baseline: 7.1629x; 1.1566x over previous
"""CRF negative-log-likelihood loss on 8 Trainium2 NeuronCores.

Data-parallel over batch (128 sequences/core) + segmented time axis with
overlapped warm-up.

The forward (log-partition) recurrence runs on device in the exp domain:

    W_{t+1} = (E' @ W_t) * exp(logits_t - C0),   E' = exp(transitions)

Products of positive matrices contract to rank-1 (Perron-Frobenius), so the
state *direction* at any t is recovered by running the recurrence over the
preceding OV steps from an arbitrary positive start.  Each length-L segment
therefore runs OV warm-up steps (over the previous segment's last OV inputs,
from an all-ones start) followed by its own L steps -- all NSEG segments in
parallel, no cross-segment communication.  Segment 0's warm-up inputs are a
START-tag indicator, which holds its state exactly proportional to the true
start vector.  bf16 W snapshots go to HBM every J steps; the host
reconstructs log Z at t=lens in f64 from the snapshot preceding it,
stitching per-segment scales by telescoped ratios anchored at the true w0.
Gold-path emission/transition scores are host-side gathers.

Per core the NSEG x 128 batch chains pack as NST streams x [128 partitions
(2 vertical bands of 52 tags), 512 columns]; each stream step is 2
concurrent quadrant matmuls (tile_position packing) + 1 DVE multiply.
"""

import numpy as np

# problem constants (hardcoded per contract)
B, T, K = 1024, 512, 52
START, STOP = 50, 51
NCORES = 8
BPC = B // NCORES          # 128 sequences per core
C0 = 5.0                   # per-step log-shift folded into exp(logits)
L = 16                     # segment length
OV = 3                     # warm-up (direction bootstrap) steps per segment
J = 8                      # snapshot interval
NSEG = T // L              # 32 segments
NST = 4                    # streams (independent instruction chains)
CW = 512                   # columns per stream
NSTEP = L + OV             # 20 device steps per chain
NSNAP = L // J + 1         # 3 snapshots (at local t-offsets 0, J, 2J)
NWARM = 24                 # PE warm-up matmuls (HAM un-throttle)

_PROG_CACHE = {}


def _build_program():
    import concourse.mybir as mybir
    import concourse.tile as tile
    from concourse import bacc

    f32 = mybir.dt.float32
    bf16 = mybir.dt.bfloat16

    nc = bacc.Bacc("TRN2", target_bir_lowering=False, debug=False,
                   num_devices=NCORES)
    # e[st, p, l, c]: step multiplier for the chain at (stream st, col c),
    # tag p%64, band p//64, local step l (l<OV: warm-up slices).
    e_d = nc.dram_tensor("e", [NST, 128, NSTEP, CW], bf16,
                         kind="ExternalInput")
    # ehat[j, i] = E'[i, j] (plus zero cols 52-63), replicated at rows 64+.
    ehat_d = nc.dram_tensor("ehat", [128, 64], bf16, kind="ExternalInput")
    snap_d = nc.dram_tensor("snap", [NST, NSNAP, 128, CW], bf16,
                            kind="ExternalOutput")

    # e chunks per stream: fine-grained so compute ramps with DMA arrival
    CH = [(a, min(a + 4, NSTEP)) for a in range(0, NSTEP, 4)]

    with tile.TileContext(nc) as tc:
        with (
            tc.tile_pool(name="const", bufs=1) as cpool,
            tc.tile_pool(name="ech", bufs=1) as epool,
            tc.tile_pool(name="w", bufs=6) as wpool,
            tc.tile_pool(name="u", bufs=1, space="PSUM") as upool,
            tc.tile_pool(name="wu", bufs=1, space="PSUM") as wupool,
        ):
            ehat = cpool.tile([128, 64], bf16, tag="ehat")
            nc.sync.dma_start(ehat[:], ehat_d[:])

            # input chunks split across both HWDGE rings (SP: even streams,
            # ACT: odd) and ordered by first use
            ech = [[None] * len(CH) for _ in range(NST)]
            for ci, (c0, c1) in enumerate(CH):
                for st in range(NST):
                    t_ = epool.tile([128, c1 - c0, CW], bf16,
                                    tag=f"e{st}_{ci}", name=f"e{st}_{ci}")
                    eng = nc.sync if st % 2 == 0 else nc.scalar
                    eng.dma_start(t_[:], e_d[st, :, c0:c1, :])
                    ech[st][ci] = t_

            # PE warm-up: dense dummy matmuls while the e DMAs stream in,
            # so HAM un-throttles the PE clock before the chains start.
            for i in range(NWARM):
                wu = wupool.tile([64, 64], f32, tag="wu", name="wu")
                nc.tensor.matmul(wu[:], ehat[0:52, 0:64], ehat[0:52, 0:64],
                                 start=True, stop=True)

            # all chains start from ones; segment 0's warm-up input mask
            # collapses its state onto the true start direction
            W = []
            for st in range(NST):
                wa = wpool.tile([128, CW], bf16, tag=f"w{st}",
                                name=f"wa{st}")
                nc.gpsimd.memset(wa[:], 1.0)
                W.append(wa)

            for el in range(NSTEP):
                for st in range(NST):
                    et = ech[st][el // 4][:, el % 4, :]
                    u = upool.tile([128, CW], f32, tag=f"u{st}",
                                   name=f"u{st}")
                    nc.tensor.matmul(u[0:64, :], ehat[0:52, :],
                                     W[st][0:52, :], start=True, stop=True)
                    nc.tensor.matmul(u[64:128, :], ehat[64:116, :],
                                     W[st][64:116, :], start=True, stop=True)
                    wn = wpool.tile([128, CW], bf16, tag=f"w{st}",
                                    name=f"wn{st}")
                    nc.vector.tensor_mul(wn[:], u[:], et)
                    W[st] = wn
                off = el + 1 - OV        # real t-offset of the new state
                if off >= 0 and off % J == 0:
                    for st in range(NST):
                        eng = nc.scalar if st % 2 == 0 else nc.sync
                        eng.dma_start(snap_d[st, off // J], W[st][:])

    nc.compile()
    return nc


def _get_program():
    if "p" not in _PROG_CACHE:
        _PROG_CACHE["p"] = _build_program()
    return _PROG_CACHE["p"]


def _to_bf16_np(x):
    import ml_dtypes
    return np.asarray(x, np.float32).astype(ml_dtypes.bfloat16)


def _host_prep(logits, trans):
    """Build per-core input maps."""
    Ep = np.exp(trans.astype(np.float64)).astype(np.float32)    # [K,K]
    ehat = np.zeros((128, 64), np.float32)
    ehat[0:K, 0:K] = Ep.T                                       # [j, i]
    ehat[64:64 + K, 0:K] = Ep.T

    ex = np.exp(logits.astype(np.float32) - C0)                 # [B,T,K]
    # [core, b, seg, l, k]
    ex_r = ex.reshape(NCORES, BPC, NSEG, L, K)
    eD = np.zeros((NCORES, NST, 128, NSTEP, CW), np.float32)
    for s in range(NSEG):
        st, v, blk = s // 8, (s % 8) // 4, s % 4
        rows = slice(64 * v, 64 * v + K)
        cols = slice(128 * blk, 128 * blk + 128)
        # [core, k, l, b]
        eD[:, st, rows, OV:, cols] = \
            ex_r[:, :, s, :, :].transpose(0, 3, 2, 1)
        if s == 0:
            # START indicator holds the state on the true start direction
            eD[:, st, 64 * v + START, 0:OV, cols] = 1.0
        else:
            eD[:, st, rows, 0:OV, cols] = \
                ex_r[:, :, s - 1, L - OV:, :].transpose(0, 3, 2, 1)

    ehat_b = _to_bf16_np(ehat)
    in_maps = []
    for c in range(NCORES):
        in_maps.append({"e": _to_bf16_np(eD[c]), "ehat": ehat_b})
    return in_maps


def _host_post(results, logits, trans, lens):
    """Reconstruct log Z at t=lens per sequence, in f64, from snapshots."""
    Ep64 = np.exp(trans.astype(np.float64))                     # [K,K]
    r64 = Ep64[STOP]                                            # [K]
    logits64 = logits.astype(np.float64)

    # y[s, k, b, :] = device state of segment s at absolute t = s*L + k*J
    y = np.zeros((NSEG, NSNAP, B, K))
    for c in range(NCORES):
        snap = np.asarray(results[c]["snap"], np.float32)
        bs = slice(c * BPC, (c + 1) * BPC)
        for s in range(NSEG):
            st, v, blk = s // 8, (s % 8) // 4, s % 4
            rows = slice(64 * v, 64 * v + K)
            cols = slice(128 * blk, 128 * blk + 128)
            for k in range(NSNAP):
                y[s, k, bs] = snap[st, k, rows, cols].T

    # telescoped segment scales, anchored at the exact start vector w0
    # (r.w0 = r[START]): W_true(s*L) = gamma_s * y[s, 0]
    lnc = np.zeros((NSEG, B))
    lnc[0] = np.log(r64[START]) - np.log(y[0, 0] @ r64)
    for s in range(1, NSEG):
        num = y[s - 1, NSNAP - 1] @ r64
        den = y[s, 0] @ r64
        lnc[s] = lnc[s - 1] + np.log(num) - np.log(den)

    t_all = lens.astype(np.int64)                               # [B], 1..512
    s_all = (t_all - 1) // L
    lpos = t_all - s_all * L                                    # 1..L
    k_all = lpos // J                                           # 0..2
    steps = lpos - k_all * J                                    # 0..J-1
    t0_all = s_all * L + k_all * J

    Wf = y[s_all, k_all, np.arange(B)]                          # [B, K]
    e64 = None
    for n in range(1, J):
        sel = steps >= n
        if not np.any(sel):
            continue
        if e64 is None:
            e64 = np.exp(logits64 - C0)
        tt = t0_all[sel] + n - 1
        Wf[sel] = (Wf[sel] @ Ep64.T) * e64[sel, tt, :]
    part = (np.log(Wf @ r64) + lnc[s_all, np.arange(B)]
            + C0 * t_all)
    return part


def _gold_scores(logits, trans, labels, lens):
    logits64 = logits.astype(np.float64)
    trans64 = trans.astype(np.float64)
    labels_ext = np.concatenate(
        [np.full((B, 1), START, np.int64), labels,
         np.full((B, 1), STOP, np.int64)], 1)
    pos = np.arange(T + 2)[None, :]
    labels_ext = np.where(pos < (lens + 1)[:, None], labels_ext, STOP)
    prev, nxt = labels_ext[:, :-1], labels_ext[:, 1:]
    m_trn = (np.arange(T + 1)[None, :] < (lens + 1)[:, None])
    transition_score = (trans64[nxt, prev] * m_trn).sum(1)
    em = np.take_along_axis(logits64, labels[:, :, None], 2)[:, :, 0]
    m_em = (np.arange(T)[None, :] < lens[:, None])
    emission_score = (em * m_em).sum(1)
    return emission_score, transition_score


def kernel(logits, transitions, labels, lens, _trace=False):
    from concourse.bass_utils import run_bass_kernel_spmd

    logits = np.asarray(logits, dtype=np.float32)
    transitions = np.asarray(transitions, dtype=np.float32)
    labels_np = np.asarray(labels).astype(np.int64)
    lens_np = np.asarray(lens).astype(np.int64)

    nc = _get_program()
    in_maps = _host_prep(logits, transitions)
    out = run_bass_kernel_spmd(nc, in_maps, list(range(NCORES)),
                               trace=_trace)
    partition = _host_post(out.results, logits, transitions, lens_np)
    emission, transition = _gold_scores(logits, transitions, labels_np,
                                        lens_np)
    loss = partition + emission - transition
    if _trace:
        kernel._last_exec_ns = out.exec_time_ns
        kernel._last_profile = out.profile_json
    return loss.astype(np.float32)
